# revision 41
# baseline (speedup 1.0000x reference)
"""Mixtral sparse-MoE block on 8 Trainium2 NeuronCores (sparse expert parallel).

Strategy: expert weights sharded along E (one expert per core). Routing is a
data-dependent shard: the host computes the (cheap, 67 MFLOP) router top-2 and
gathers each expert's routed tokens to a fixed capacity C, so each core runs
its expert's up/gate/down matmuls (99.9% of the module's FLOPs) only on the
~T*K/E tokens actually routed to it instead of all T tokens — a ~3.6x FLOP
reduction versus the dense-masked formulation. Each core scales its output by
the renormalized top-2 combine weight on-chip; the host scatter-adds the two
expert contributions per token (the psum over the expert axis).

On-chip everything is feature-major: all three matmuls keep weights as the
stationary operand and tokens as the moving free dim, zero on-chip transposes.
Matmuls run in bf16 (same PE rate as fp32r on TRN2, half the DMA/SBUF), with
fp32 PSUM accumulation; verified rel_max error ~5.6e-3 on hardware, well
inside the 2e-2 gate (fp8 was tested numerically and fails at 5.6e-2).

Layout details tuned against the cost model: up/gate weights are interleaved
per f-chunk on the host ([H, FC, {up,gate}, 128]) so each weight DMA moves
512B-contiguous runs (full DMA bus efficiency) and one descriptor-gen slot
feeds both matmuls; gathered activations are chunk-major (one DRAM tensor per
512-token chunk) so the first chunk streams in row-pieces at the rate the
first accumulation groups consume it while later chunks arrive as two slabs;
per-h-chunk output stores for chunks >=1 merge into one SBUF row tile flushed
by a single DMA (one descriptor-gen slot on the drain tail, not two); the
combine-weight broadcast uses the gpsimd partition-broadcast path (no PE or
DVE time) and is emitted off the pipeline-start critical path.
"""

import numpy as np
import ml_dtypes

import concourse.mybir as mybir
import concourse.tile as tile
from concourse import bacc
from concourse.bass_utils import run_bass_kernel_spmd

# Problem shape (hardcoded per contract).
B, S, H, F, E = 2, 2048, 1024, 2048, 8
T = B * S                    # 4096 tokens
N_CORES = 8
HC = H // 128                # 8 h-chunks
FC = F // 128                # 16 f-chunks

f32 = mybir.dt.float32
f32r = mybir.dt.float32r
bf16 = mybir.dt.bfloat16
AF = mybir.ActivationFunctionType
BF16_NP = ml_dtypes.bfloat16


def _col_chunks(C):
    out, o = [], 0
    while o < C:
        n = min(512, C - o)
        out.append((o, n))
        o += n
    return out


def build(C):
    """One expert on C gathered tokens: y[:, t] = cw[t] * FFN(xg[:, t])."""
    nc = bacc.Bacc("TRN2", target_bir_lowering=False, debug=False,
                   num_devices=N_CORES)

    cols = _col_chunks(C)

    # Activations arrive as one DRAM tensor per 512-token chunk (chunk-major
    # host layout): the first chunk streams per h-chunk row to feed the first
    # matmul group as early as possible; later chunks land as single
    # full-bandwidth slabs while the PE is already saturated.
    xgs = [nc.dram_tensor(f"xg{k}", [H, n], bf16, kind="ExternalInput").ap()
           for k, (o, n) in enumerate(cols)]
    wug = nc.dram_tensor("wug", [H, 2 * F], bf16, kind="ExternalInput").ap()
    wd = nc.dram_tensor("wd", [F, H], bf16, kind="ExternalInput").ap()
    cw = nc.dram_tensor("cw", [1, C], f32, kind="ExternalInput").ap()
    y = nc.dram_tensor("y", [H, C], bf16, kind="ExternalOutput").ap()

    # DRAM views with the 128-partition dim pulled out front.
    xg_vs = [t.rearrange("(hc p) n -> p hc n", p=128) for t in xgs]
    wug_v = wug.rearrange("(hc p) (fc g) -> p hc fc g",
                          p=128, g=256)                       # [128, 8, 16, 256]
    wd_v = wd.rearrange("(fc p) h -> p fc h", p=128)          # [128, 16, 1024]

    with tile.TileContext(nc) as tc:
        with (
            tc.tile_pool(name="const", bufs=1) as cpool,
            tc.tile_pool(name="w", bufs=2) as wpool,
            tc.tile_pool(name="inner", bufs=1) as ipool,
            tc.tile_pool(name="work", bufs=3) as spool,
            tc.tile_pool(name="psUp", bufs=2, space="PSUM") as psUp,
            tc.tile_pool(name="psGt", bufs=3, space="PSUM") as psGt,
            tc.tile_pool(name="psY", bufs=3, space="PSUM") as psY,
        ):
            # DMA order tuned for pipeline start: the first matmul group needs
            # wug[fc=0] and chunk-0 activations only, streamed per h-chunk at
            # the rate the first accumulation groups consume them; later
            # chunks follow as whole slabs.
            n0 = cols[0][1]
            xg_sb = cpool.tile([128, HC, C], bf16)
            wug_ts = [None] * FC
            wug_ts[0] = wpool.tile([128, HC, 256], bf16, tag="wug",
                                   name="wug_t0")
            nc.sync.dma_start(wug_ts[0][:, 0, :], wug_v[:, 0, 0, :])
            nc.sync.dma_start(xg_sb[:, 0:2, 0:n0], xg_vs[0][:, 0:2, :])
            nc.sync.dma_start(wug_ts[0][:, 1:HC, :], wug_v[:, 1:HC, 0, :])
            nc.sync.dma_start(xg_sb[:, 2:5, 0:n0], xg_vs[0][:, 2:5, :])
            nc.sync.dma_start(xg_sb[:, 5:HC, 0:n0], xg_vs[0][:, 5:HC, :])

            def xg_ap(hc, o, n):
                return xg_sb[:, hc, o:o + n]

            def wug_ap(wug_t, fc, hc, half):
                return wug_t[:, hc, half * 128:(half + 1) * 128]
            # Later chunks and the fc=1 weights interleave so neither the
            # chunk-1 matmuls nor the fc=1 groups wait on the DMA line.
            if len(cols) > 1:
                o, n = cols[1]
                nc.sync.dma_start(xg_sb[:, 0:4, o:o + n], xg_vs[1][:, 0:4, :])
                nc.sync.dma_start(xg_sb[:, 4:HC, o:o + n],
                                  xg_vs[1][:, 4:HC, :])
            if FC > 1:
                wug_ts[1] = wpool.tile([128, HC, 256], bf16, tag="wug",
                                       name="wug_t1")
                nc.sync.dma_start(wug_ts[1][:], wug_v[:, :, 1, :])
            for k in range(2, len(cols)):
                o, n = cols[k]
                nc.sync.dma_start(xg_sb[:, :, o:o + n], xg_vs[k][:])
            cw_sb = cpool.tile([1, C], f32)
            nc.sync.dma_start(cw_sb[:], cw[:])

            cwb = cpool.tile([128, C], f32)

            # ---- Phase A: up/gate matmuls + silu -> inner (bf16) ----
            inner = []
            for fc in range(FC):
                wug_t = wug_ts[fc]
                if wug_t is None:
                    wug_t = wpool.tile([128, HC, 256], bf16, tag="wug")
                    nc.sync.dma_start(wug_t[:], wug_v[:, :, fc, :])
                it = ipool.tile([128, C], bf16, tag=f"inner{fc}")
                for (o, n) in cols:
                    up_ps = psUp.tile([128, 512], f32, tag="up")
                    for hc in range(HC):
                        nc.tensor.matmul(up_ps[:, :n],
                                         wug_ap(wug_t, fc, hc, 0),
                                         xg_ap(hc, o, n),
                                         start=(hc == 0), stop=(hc == HC - 1))
                    gt_ps = psGt.tile([128, 512], f32, tag="gt")
                    for hc in range(HC):
                        nc.tensor.matmul(gt_ps[:, :n],
                                         wug_ap(wug_t, fc, hc, 1),
                                         xg_ap(hc, o, n),
                                         start=(hc == 0), stop=(hc == HC - 1))
                    silu = spool.tile([128, 512], f32, tag="silu")
                    nc.scalar.activation(silu[:, :n], up_ps[:, :n], AF.Silu)
                    nc.vector.tensor_mul(it[:, o:o + n], silu[:, :n],
                                         gt_ps[:, :n])
                inner.append(it)
                if fc == 0:
                    # Combine-weight broadcast to [128, C] on the gpsimd DMA
                    # path — costs no PE/DVE time; emitted here (not at t=0)
                    # so it doesn't gate the pipeline start, and well before
                    # phase B needs cwb.
                    nc.gpsimd.partition_broadcast(cwb[:], cw_sb[:])

            # ---- Phase B: down matmul + combine scale ----
            for hc in range(HC):
                hs = slice(hc * 128, (hc + 1) * 128)
                if hc % 2 == 0:
                    # 2 h-chunks per DMA: 512B-contiguous runs.
                    wd_t = wpool.tile([128, FC, 256], bf16, tag="wd")
                    nc.sync.dma_start(wd_t[:],
                                      wd_v[:, :, hc * 128:(hc + 2) * 128])
                ho = (hc % 2) * 128
                # Chunk 0 stores on its own; later chunks write one merged
                # SBUF row tile flushed by a single DMA, so the final h-chunk
                # pays one store slot after the last DVE mul, not two
                # serialized ones.
                ym = None
                for k, (o, n) in enumerate(cols):
                    y_ps = psY.tile([128, 512], f32, tag="y")
                    for fc in range(FC):
                        nc.tensor.matmul(y_ps[:, :n], wd_t[:, fc, ho:ho + 128],
                                         inner[fc][:, o:o + n],
                                         start=(fc == 0), stop=(fc == FC - 1))
                    if k == 0:
                        y_sb = spool.tile([128, 512], bf16, tag="ysb")
                        nc.vector.tensor_mul(y_sb[:, :n], y_ps[:, :n],
                                             cwb[:, o:o + n])
                        nc.sync.dma_start(y[hs, o:o + n], y_sb[:, :n])
                    else:
                        if ym is None:
                            ym = spool.tile([128, max(C - 512, 1)], bf16,
                                            tag="ym", name="ym")
                        nc.vector.tensor_mul(ym[:, o - 512:o - 512 + n],
                                             y_ps[:, :n], cwb[:, o:o + n])
                if ym is not None:
                    nc.sync.dma_start(y[hs, 512:C], ym[:])

    nc.compile()
    return nc


_CACHED = {}


def _get_program(C=1096):
    if C not in _CACHED:
        _CACHED[C] = build(C)
    return _CACHED[C]


def _route(x, gw):
    """Host router: top-2 selection + renormalized softmax weights.

    Softmax is monotonic so top-2 of the probs == top-2 of the logits; the
    renormalized weights depend only on the top-2 logits:
    w_i = exp(l_i) / (exp(l_1) + exp(l_2)).
    """
    logits = (x @ gw).astype(np.float64)                  # [T, E]
    sel = np.argsort(-logits, axis=-1, kind="stable")[:, :2]
    l12 = np.take_along_axis(logits, sel, axis=-1)
    z = np.exp(l12 - l12[:, :1])
    w = z / z.sum(-1, keepdims=True)
    return sel, w.astype(np.float32)


def kernel(hidden_states, gate_w, w_up, w_gate, w_down):
    x = np.asarray(hidden_states, np.float32).reshape(T, H)
    gw = np.asarray(gate_w, np.float32)
    sel, w = _route(x, gw)

    idxs, cws = [], []
    for e in range(E):
        m0 = sel[:, 0] == e
        m1 = sel[:, 1] == e
        te = np.nonzero(m0 | m1)[0]
        we = np.where(m0[te], w[te, 0], w[te, 1])
        idxs.append(te)
        cws.append(we)

    cap = max(len(t) for t in idxs)
    C = max(256, -(-cap // 8) * 8)
    nc = _get_program(C)

    in_maps = []
    for c in range(N_CORES):
        n = len(idxs[c])
        xgT = np.zeros((C, H), BF16_NP)
        xgT[:n] = x[idxs[c]].astype(BF16_NP)
        xgf = np.ascontiguousarray(xgT.T)                 # [H, C]
        cwp = np.zeros((1, C), np.float32)
        cwp[0, :n] = cws[c]
        wug = np.empty((H, FC, 2, 128), BF16_NP)
        wug[:, :, 0, :] = np.asarray(w_up[c], np.float32).astype(
            BF16_NP).reshape(H, FC, 128)
        wug[:, :, 1, :] = np.asarray(w_gate[c], np.float32).astype(
            BF16_NP).reshape(H, FC, 128)
        im = {
            "wug": wug.reshape(H, 2 * F),
            "wd": np.asarray(w_down[c], np.float32).astype(BF16_NP),
            "cw": cwp,
        }
        for k, (o, nn) in enumerate(_col_chunks(C)):
            im[f"xg{k}"] = np.ascontiguousarray(xgf[:, o:o + nn])
        in_maps.append(im)
    res = run_bass_kernel_spmd(nc, in_maps, list(range(N_CORES)))

    y = np.zeros((T, H), np.float32)
    for c in range(N_CORES):
        n = len(idxs[c])
        yc = np.asarray(res.results[c]["y"], np.float32)  # [H, C]
        y[idxs[c]] += yc[:, :n].T
    return y.reshape(B, S, H)


# revision 48
# speedup vs baseline: 1.0047x; 1.0047x over previous
"""Mixtral sparse-MoE block on 8 Trainium2 NeuronCores (sparse expert parallel).

Strategy: expert weights sharded along E (one expert per core). Routing is a
data-dependent shard: the host computes the (cheap, 67 MFLOP) router top-2 and
gathers each expert's routed tokens to a fixed capacity C, so each core runs
its expert's up/gate/down matmuls (99.9% of the module's FLOPs) only on the
~T*K/E tokens actually routed to it instead of all T tokens — a ~3.6x FLOP
reduction versus the dense-masked formulation. Each core scales its output by
the renormalized top-2 combine weight on-chip; the host scatter-adds the two
expert contributions per token (the psum over the expert axis).

On-chip everything is feature-major: all three matmuls keep weights as the
stationary operand and tokens as the moving free dim, zero on-chip transposes.
Matmuls run in bf16 (same PE rate as fp32r on TRN2, half the DMA/SBUF), with
fp32 PSUM accumulation; verified rel_max error ~5.6e-3 on hardware, well
inside the 2e-2 gate (fp8 was tested numerically and fails at 5.6e-2).

Layout details tuned against the cost model: up/gate weights are interleaved
per f-chunk on the host ([H, FC, {up,gate}, 128]) so each weight DMA moves
512B-contiguous runs (full DMA bus efficiency) and one descriptor-gen slot
feeds both matmuls; gathered activations are chunk-major (one DRAM tensor per
512-token chunk) so the first chunk streams in row-pieces at the rate the
first accumulation groups consume it while later chunks arrive as two slabs;
per-h-chunk output stores for chunks >=1 merge into one SBUF row tile flushed
by a single DMA (one descriptor-gen slot on the drain tail, not two); the
combine-weight broadcast uses the gpsimd partition-broadcast path (no PE or
DVE time) and is emitted off the pipeline-start critical path.
"""

import numpy as np
import ml_dtypes

import concourse.mybir as mybir
import concourse.tile as tile
from concourse import bacc
from concourse.bass_utils import run_bass_kernel_spmd

# Problem shape (hardcoded per contract).
B, S, H, F, E = 2, 2048, 1024, 2048, 8
T = B * S                    # 4096 tokens
N_CORES = 8
HC = H // 128                # 8 h-chunks
FC = F // 128                # 16 f-chunks

f32 = mybir.dt.float32
f32r = mybir.dt.float32r
bf16 = mybir.dt.bfloat16
AF = mybir.ActivationFunctionType
BF16_NP = ml_dtypes.bfloat16


def _col_chunks(C):
    out, o = [], 0
    while o < C:
        n = min(512, C - o)
        out.append((o, n))
        o += n
    return out


def build(C):
    """One expert on C gathered tokens: y[:, t] = cw[t] * FFN(xg[:, t])."""
    nc = bacc.Bacc("TRN2", target_bir_lowering=False, debug=False,
                   num_devices=N_CORES)

    cols = _col_chunks(C)

    # Activations arrive chunk-major: the first 512-token chunk as its own
    # DRAM tensor streamed in row-pieces to feed the first matmul groups as
    # early as possible, and all remaining tokens as one remainder tensor
    # loaded in two full-bandwidth halves while the PE is already saturated.
    nc0 = cols[0][1]
    xg0 = nc.dram_tensor("xg0", [H, nc0], bf16, kind="ExternalInput").ap()
    xgR = None
    if C > nc0:
        xgR = nc.dram_tensor("xgR", [H, C - nc0], bf16,
                             kind="ExternalInput").ap()
    wug = nc.dram_tensor("wug", [H, 2 * F], bf16, kind="ExternalInput").ap()
    wd = nc.dram_tensor("wd", [F, H], bf16, kind="ExternalInput").ap()
    cw = nc.dram_tensor("cw", [1, C], f32, kind="ExternalInput").ap()
    y = nc.dram_tensor("y", [H, C], bf16, kind="ExternalOutput").ap()

    # DRAM views with the 128-partition dim pulled out front.
    xg0_v = xg0.rearrange("(hc p) n -> p hc n", p=128)
    xgR_v = xgR.rearrange("(hc p) n -> p hc n", p=128) if xgR else None
    wug_v = wug.rearrange("(hc p) (fc g) -> p hc fc g",
                          p=128, g=256)                       # [128, 8, 16, 256]
    wd_v = wd.rearrange("(fc p) h -> p fc h", p=128)          # [128, 16, 1024]

    with tile.TileContext(nc) as tc:
        with (
            tc.tile_pool(name="const", bufs=1) as cpool,
            tc.tile_pool(name="w", bufs=2) as wpool,
            tc.tile_pool(name="inner", bufs=1) as ipool,
            tc.tile_pool(name="work", bufs=3) as spool,
            tc.tile_pool(name="psUp", bufs=2, space="PSUM") as psUp,
            tc.tile_pool(name="psGt", bufs=3, space="PSUM") as psGt,
            tc.tile_pool(name="psY", bufs=3, space="PSUM") as psY,
        ):
            # DMA order tuned for pipeline start: the first matmul group needs
            # wug[fc=0] and chunk-0 activations only, streamed per h-chunk at
            # the rate the first accumulation groups consume them; later
            # chunks follow as whole slabs.
            n0 = cols[0][1]
            xg_sb = cpool.tile([128, HC, C], bf16)
            wug_ts = [None] * FC
            wug_ts[0] = wpool.tile([128, HC, 256], bf16, tag="wug",
                                   name="wug_t0")
            nc.sync.dma_start(wug_ts[0][:, 0, :], wug_v[:, 0, 0, :])
            nc.sync.dma_start(xg_sb[:, 0:2, 0:n0], xg0_v[:, 0:2, :])
            nc.sync.dma_start(wug_ts[0][:, 1:5, :], wug_v[:, 1:5, 0, :])
            nc.sync.dma_start(xg_sb[:, 2:5, 0:n0], xg0_v[:, 2:5, :])
            nc.sync.dma_start(wug_ts[0][:, 5:HC, :], wug_v[:, 5:HC, 0, :])
            nc.sync.dma_start(xg_sb[:, 5:HC, 0:n0], xg0_v[:, 5:HC, :])

            def xg_ap(hc, o, n):
                return xg_sb[:, hc, o:o + n]

            def wug_ap(wug_t, fc, hc, half):
                return wug_t[:, hc, half * 128:(half + 1) * 128]
            # The token remainder and the fc=1 weights interleave so neither
            # the chunk-1 matmuls nor the fc=1 groups wait on the DMA line.
            if xgR is not None:
                nc.sync.dma_start(xg_sb[:, 0:4, n0:C], xgR_v[:, 0:4, :])
                nc.sync.dma_start(xg_sb[:, 4:HC, n0:C], xgR_v[:, 4:HC, :])
            if FC > 1:
                wug_ts[1] = wpool.tile([128, HC, 256], bf16, tag="wug",
                                       name="wug_t1")
                nc.sync.dma_start(wug_ts[1][:], wug_v[:, :, 1, :])
            cw_sb = cpool.tile([1, C], f32)
            nc.sync.dma_start(cw_sb[:], cw[:])

            cwb = cpool.tile([128, C], f32)

            # ---- Phase A: up/gate matmuls + silu -> inner (bf16) ----
            inner = []
            for fc in range(FC):
                wug_t = wug_ts[fc]
                if wug_t is None:
                    wug_t = wpool.tile([128, HC, 256], bf16, tag="wug")
                    nc.sync.dma_start(wug_t[:], wug_v[:, :, fc, :])
                it = ipool.tile([128, C], bf16, tag=f"inner{fc}")
                for (o, n) in cols:
                    up_ps = psUp.tile([128, 512], f32, tag="up")
                    for hc in range(HC):
                        nc.tensor.matmul(up_ps[:, :n],
                                         wug_ap(wug_t, fc, hc, 0),
                                         xg_ap(hc, o, n),
                                         start=(hc == 0), stop=(hc == HC - 1))
                    gt_ps = psGt.tile([128, 512], f32, tag="gt")
                    for hc in range(HC):
                        nc.tensor.matmul(gt_ps[:, :n],
                                         wug_ap(wug_t, fc, hc, 1),
                                         xg_ap(hc, o, n),
                                         start=(hc == 0), stop=(hc == HC - 1))
                    silu = spool.tile([128, 512], f32, tag="silu")
                    nc.scalar.activation(silu[:, :n], up_ps[:, :n], AF.Silu)
                    nc.vector.tensor_mul(it[:, o:o + n], silu[:, :n],
                                         gt_ps[:, :n])
                inner.append(it)
                if fc == 0:
                    # Combine-weight broadcast to [128, C] on the gpsimd DMA
                    # path — costs no PE/DVE time; emitted here (not at t=0)
                    # so it doesn't gate the pipeline start, and well before
                    # phase B needs cwb.
                    nc.gpsimd.partition_broadcast(cwb[:], cw_sb[:])

            # ---- Phase B: down matmul + combine scale ----
            for hc in range(HC):
                hs = slice(hc * 128, (hc + 1) * 128)
                if hc % 2 == 0:
                    # 2 h-chunks per DMA: 512B-contiguous runs.
                    wd_t = wpool.tile([128, FC, 256], bf16, tag="wd")
                    nc.sync.dma_start(wd_t[:],
                                      wd_v[:, :, hc * 128:(hc + 2) * 128])
                ho = (hc % 2) * 128
                # Chunk 0 stores on its own; later chunks write one merged
                # SBUF row tile flushed by a single DMA, so the final h-chunk
                # pays one store slot after the last DVE mul, not two
                # serialized ones.
                ym = None
                for k, (o, n) in enumerate(cols):
                    y_ps = psY.tile([128, 512], f32, tag="y")
                    for fc in range(FC):
                        nc.tensor.matmul(y_ps[:, :n], wd_t[:, fc, ho:ho + 128],
                                         inner[fc][:, o:o + n],
                                         start=(fc == 0), stop=(fc == FC - 1))
                    if k == 0:
                        y_sb = spool.tile([128, 512], bf16, tag="ysb")
                        nc.vector.tensor_mul(y_sb[:, :n], y_ps[:, :n],
                                             cwb[:, o:o + n])
                        nc.sync.dma_start(y[hs, o:o + n], y_sb[:, :n])
                    else:
                        if ym is None:
                            ym = spool.tile([128, max(C - 512, 1)], bf16,
                                            tag="ym", name="ym")
                        nc.vector.tensor_mul(ym[:, o - 512:o - 512 + n],
                                             y_ps[:, :n], cwb[:, o:o + n])
                if ym is not None:
                    nc.sync.dma_start(y[hs, 512:C], ym[:])

    nc.compile()
    return nc


_CACHED = {}


def _get_program(C=1092):
    if C not in _CACHED:
        _CACHED[C] = build(C)
    return _CACHED[C]


def _route(x, gw):
    """Host router: top-2 selection + renormalized softmax weights.

    Softmax is monotonic so top-2 of the probs == top-2 of the logits; the
    renormalized weights depend only on the top-2 logits:
    w_i = exp(l_i) / (exp(l_1) + exp(l_2)).
    """
    logits = (x @ gw).astype(np.float64)                  # [T, E]
    sel = np.argsort(-logits, axis=-1, kind="stable")[:, :2]
    l12 = np.take_along_axis(logits, sel, axis=-1)
    z = np.exp(l12 - l12[:, :1])
    w = z / z.sum(-1, keepdims=True)
    return sel, w.astype(np.float32)


def kernel(hidden_states, gate_w, w_up, w_gate, w_down):
    x = np.asarray(hidden_states, np.float32).reshape(T, H)
    gw = np.asarray(gate_w, np.float32)
    sel, w = _route(x, gw)

    idxs, cws = [], []
    for e in range(E):
        m0 = sel[:, 0] == e
        m1 = sel[:, 1] == e
        te = np.nonzero(m0 | m1)[0]
        we = np.where(m0[te], w[te, 0], w[te, 1])
        idxs.append(te)
        cws.append(we)

    cap = max(len(t) for t in idxs)
    C = max(256, -(-cap // 4) * 4)
    nc = _get_program(C)

    in_maps = []
    for c in range(N_CORES):
        n = len(idxs[c])
        xgT = np.zeros((C, H), BF16_NP)
        xgT[:n] = x[idxs[c]].astype(BF16_NP)
        xgf = np.ascontiguousarray(xgT.T)                 # [H, C]
        cwp = np.zeros((1, C), np.float32)
        cwp[0, :n] = cws[c]
        wug = np.empty((H, FC, 2, 128), BF16_NP)
        wug[:, :, 0, :] = np.asarray(w_up[c], np.float32).astype(
            BF16_NP).reshape(H, FC, 128)
        wug[:, :, 1, :] = np.asarray(w_gate[c], np.float32).astype(
            BF16_NP).reshape(H, FC, 128)
        im = {
            "wug": wug.reshape(H, 2 * F),
            "wd": np.asarray(w_down[c], np.float32).astype(BF16_NP),
            "cw": cwp,
            "xg0": np.ascontiguousarray(xgf[:, 0:min(512, C)]),
        }
        if C > 512:
            im["xgR"] = np.ascontiguousarray(xgf[:, 512:C])
        in_maps.append(im)
    res = run_bass_kernel_spmd(nc, in_maps, list(range(N_CORES)))

    y = np.zeros((T, H), np.float32)
    for c in range(N_CORES):
        n = len(idxs[c])
        yc = np.asarray(res.results[c]["y"], np.float32)  # [H, C]
        y[idxs[c]] += yc[:, :n].T
    return y.reshape(B, S, H)
